# revision 35
# baseline (speedup 1.0000x reference)
"""Trainium2 Bass kernel for multi-head attention with RoPE (nn_Attention).

Reference computation (B=1, N=2048, D=1024, 16 heads, hd=64):
    q = x @ wq.T; k = x @ wk.T; v = x @ wv.T      (reshaped to heads)
    q, k = rope(q), rope(k)
    out = softmax(q k^T / sqrt(hd)) v              (non-causal, full)
    return (out reshaped) @ wp.T

Sharding: tensor-parallel over heads — each of the 8 cores owns 2 heads for
QKV projection + SDPA, then an AllToAll redistributes the attention output
so each core computes the final projection for its 256 sequence rows with
the full wp.

v2: all matmul operands bf16 (host-precast), x streamed in 4 column blocks
so attention starts after the first block, ScalarE reserved exclusively for
exp, V transposed via the DMA xbar, wp prefetched during attention.

Self-contained: only imports numpy + the concourse stack available in the
execution environment. kernel(**inputs) takes the full unsharded inputs and
returns the full output.
"""
import numpy as np

DIM = 1024
NHEADS = 16
HD = 64
SEQ = 2048
NCORES = 8
ROPE_BASE = 10000.0
HPC = NHEADS // NCORES      # heads per core = 2
CH = HPC * HD               # channels per core = 128
QCH = 512                   # q-chunk (free dim of S/P tiles)
NQC = SEQ // QCH            # 4
NKT = SEQ // 128            # 16 k-tiles
DCH = DIM // 128            # 8 contraction chunks
NBLK = 4                    # x column blocks (512 seq cols each)

_CACHE = {}


def _rope_tables():
    inv = 1.0 / (ROPE_BASE ** (np.arange(0, HD, 2, dtype=np.float64) / HD))
    t = np.arange(SEQ, dtype=np.float64)
    freqs = np.outer(t, inv)                      # [SEQ, 32]
    emb = np.concatenate([freqs, freqs], 1)       # [SEQ, 64]
    cosT = np.cos(emb).T                          # [64, SEQ]
    sinT = np.sin(emb).T
    sig = (np.arange(HD) + 32) % HD
    sT = sinT[sig]                                # shifted sin
    cos2 = np.concatenate([cosT, cosT], 0)        # [128, SEQ] (2 heads)
    s2 = np.concatenate([sT, sT], 0)
    return cos2, s2


def _r2t():
    # rotate-half matrix R (per head), block-diagonal over the 2 heads; we
    # pass R2.T as the stationary matmul operand.
    R = np.zeros((HD, HD), np.float64)
    for j in range(32):
        R[j, j + 32] = -1.0
        R[j + 32, j] = 1.0
    R2 = np.zeros((CH, CH), np.float64)
    R2[0:HD, 0:HD] = R
    R2[HD:CH, HD:CH] = R
    return np.ascontiguousarray(R2.T)


def _build(nrep=1, n_cores=NCORES, with_c=True, parts="real"):
    import concourse.mybir as mybir
    import concourse.tile as tile
    from concourse import bacc
    from concourse.masks import make_identity

    F32 = mybir.dt.float32
    F32R = mybir.dt.float32r
    BF16 = mybir.dt.bfloat16
    EXP = mybir.ActivationFunctionType.Exp

    nc = bacc.Bacc("TRN2", target_bir_lowering=False, debug=False,
                   num_devices=n_cores)

    xt_ext = nc.dram_tensor("xt", [DIM, SEQ], BF16, kind="ExternalInput")
    wq_ext = nc.dram_tensor("wq_t", [128, DCH * CH], BF16,
                            kind="ExternalInput")
    wk_ext = nc.dram_tensor("wk_t", [128, DCH * CH], BF16,
                            kind="ExternalInput")
    wv_ext = nc.dram_tensor("wv_t", [128, DCH * CH], BF16,
                            kind="ExternalInput")
    wp_ext = nc.dram_tensor("wp_t", [128, DCH * DIM], BF16,
                            kind="ExternalInput")
    ck_ext = nc.dram_tensor("cos_k", [CH, SEQ], BF16, kind="ExternalInput")
    sk_ext = nc.dram_tensor("sin_k", [CH, SEQ], BF16, kind="ExternalInput")
    r2t_ext = nc.dram_tensor("r2t", [CH, CH], BF16, kind="ExternalInput")
    sel_ext = nc.dram_tensor("sel", [NCORES, NHEADS, 128], BF16,
                             kind="ExternalInput")
    out_ext = nc.dram_tensor("out", [SEQ // NCORES, DIM], F32,
                             kind="ExternalOutput")
    # Collective payload is bf16 on the wire, but the collective machinery
    # mishandles sub-4-byte dtypes, so the DRAM tensors are declared f32
    # (half the elements) and DMAs bitcast at the boundary. The sequence
    # axis is host-permuted so each q-chunk contains 64 rows for EVERY
    # receiver; one small AllToAll runs per chunk, hiding the output
    # projection under later chunks' attention. Each (dst, head) slot
    # carries the unnormalized O (64 rows) plus the softmax denominator
    # row (row 64); the receiver normalizes.
    a2a_ins = [nc.dram_tensor(f"a2a_in{qc}", [NCORES, HPC, HD + 1, 32], F32)
               for qc in range(NQC)]
    a2a_outs = [nc.dram_tensor(f"a2a_out{qc}", [NCORES, HPC, HD + 1, 32], F32)
                for qc in range(NQC)]

    with tile.TileContext(nc) as tc:

        def stage_ab(Qp, Kp, Vsb, sel, xt, wp, parts="ab"):
            with (
                tc.tile_pool(name="stA", bufs=1) as A_sb,
                tc.tile_pool(name="stA2", bufs=2) as A_db,
                tc.tile_pool(name="psBig", bufs=2, space="PSUM") as psBig,
                tc.tile_pool(name="psSm", bufs=2, space="PSUM") as psSm,
                tc.tile_pool(name="psO", bufs=2, space="PSUM") as psO,
                tc.tile_pool(name="stB", bufs=4) as B_db,
                tc.tile_pool(name="stBs", bufs=3) as B_sm,
            ):
                aux1 = A_sb.tile([128, HD], F32, tag="aux1")
                nc.vector.memset(aux1[:], 1.0)
                identf = A_sb.tile([128, 128], F32, tag="identf")
                make_identity(nc, identf[:])
                ident = A_sb.tile([128, 128], BF16, tag="ident")
                nc.vector.tensor_copy(ident[:], identf[:])
                # warm the ACT exp table at t=0 so the first real exp
                # doesn't eat the ~2.7us table load.
                warm = A_sb.tile([1, 8], F32, tag="warm")
                nc.scalar.activation(out=warm[:], in_=aux1[0:1, 0:8], func=EXP)
                nc.vector.tensor_copy(
                    Vsb[:, :, :, HD],
                    aux1[:, 0:NKT * HPC].rearrange("p (k h) -> p k h", h=HPC))

                # ---- input DMAs (all bf16, single stream on sync queue),
                # ordered so the block-0 critical path (wk, xt block 0,
                # rope tables, wq) lands first.
                wk = A_sb.tile([128, DCH, CH], BF16, tag="wk")
                wq = A_sb.tile([128, DCH, CH], BF16, tag="wq")
                wv = A_sb.tile([128, DCH, CH], BF16, tag="wv")
                r2t = A_sb.tile([CH, CH], BF16, tag="r2t")
                ck = A_sb.tile([CH, SEQ], BF16, tag="ck")
                sk = A_sb.tile([CH, SEQ], BF16, tag="sk")
                xt_r = xt_ext.rearrange("(c p) n -> p c n", p=128)
                b0 = slice(0, QCH)
                nc.sync.dma_start(
                    out=wq[:], in_=wq_ext.rearrange("p (c j) -> p c j", j=CH))
                nc.sync.dma_start(
                    out=wk[:], in_=wk_ext.rearrange("p (c j) -> p c j", j=CH))
                for d in range(DCH):
                    nc.sync.dma_start(out=xt[:, d, b0], in_=xt_r[:, d, b0])
                nc.sync.dma_start(out=r2t[:], in_=r2t_ext[:])
                if wp is not None:
                    nc.gpsimd.dma_start(
                        out=sel[:], in_=sel_ext.rearrange("s r p -> r s p"))
                nc.sync.dma_start(out=sk[:, b0], in_=sk_ext[:, b0])
                nc.sync.dma_start(out=ck[:, b0], in_=ck_ext[:, b0])
                nc.sync.dma_start(
                    out=wv[:], in_=wv_ext.rearrange("p (c j) -> p c j", j=CH))
                rest = slice(QCH, SEQ)
                nc.sync.dma_start(out=sk[:, rest], in_=sk_ext[:, rest])
                nc.sync.dma_start(out=ck[:, rest], in_=ck_ext[:, rest])
                for b in range(1, NBLK):
                    sl = slice(b * QCH, (b + 1) * QCH)
                    nc.sync.dma_start(out=xt[:, :, sl], in_=xt_r[:, :, sl])
                if wp is not None:
                    # prefetch wp for stage C behind the x blocks, on the
                    # gpsimd queue so it doesn't delay the x streams
                    nc.gpsimd.dma_start(
                        out=wp[:],
                        in_=wp_ext.rearrange("p (s o) -> p s o", o=DIM))

                def rope_to(acc_ps, dst, sl, width):
                    # dst[:, sl] = acc*cos + R2T @ (acc*sin), overwriting
                    # acc_ps in place for the rotation matmul.
                    qs = A_db.tile([CH, width], BF16, tag="qs")
                    nc.vector.tensor_mul(qs[:], acc_ps[:], sk[:, sl])
                    qct = A_db.tile([CH, width], BF16, tag="qct")
                    nc.vector.tensor_mul(qct[:], acc_ps[:], ck[:, sl])
                    nc.tensor.matmul(acc_ps[:], r2t[:], qs[:],
                                     start=True, stop=True)
                    nc.vector.tensor_add(dst[:, sl], qct[:], acc_ps[:])

                def proj_units(w_sb, dst, b):
                    # split a 1024-contraction projection + rope into 3
                    # emission units so it can be paced between S steps.
                    sl = slice(b * QCH, (b + 1) * QCH)
                    box = {}

                    def u0():
                        box["ps"] = psSm.tile([CH, QCH], F32, tag="sm", name="ps")
                        for d in range(4):
                            nc.tensor.matmul(box["ps"][:], w_sb[:, d, :],
                                             xt[:, d, sl],
                                             start=(d == 0), stop=False)

                    def u1():
                        for d in range(4, DCH):
                            nc.tensor.matmul(box["ps"][:], w_sb[:, d, :],
                                             xt[:, d, sl],
                                             start=False, stop=(d == DCH - 1))

                    def u2():
                        rope_to(box["ps"], dst, sl, QCH)

                    return [u0, u1, u2]

                def v_units(b):
                    sl = slice(b * QCH, (b + 1) * QCH)
                    box = {}

                    def u0():
                        box["ps"] = psSm.tile([CH, QCH], F32, tag="sm", name="ps")
                        for d in range(4):
                            nc.tensor.matmul(box["ps"][:], wv[:, d, :],
                                             xt[:, d, sl],
                                             start=(d == 0), stop=False)

                    def u1():
                        for d in range(4, DCH):
                            nc.tensor.matmul(box["ps"][:], wv[:, d, :],
                                             xt[:, d, sl],
                                             start=False, stop=(d == DCH - 1))
                        vt = B_db.tile([CH, QCH], BF16, tag="vt",
                                       name="vt")
                        nc.vector.tensor_copy(vt[:], box["ps"][:])
                        box["vt"] = vt

                    def tr(i):
                        def u():
                            kti = b * (QCH // 128) + i
                            ps_t = psSm.tile([CH, QCH], F32, tag="sm",
                                             name="ps_t")
                            ps_tb = ps_t.bitcast(BF16)
                            nc.tensor.transpose(
                                ps_tb[:, 0:128],
                                box["vt"][:, i * 128:(i + 1) * 128],
                                ident[:])
                            nc.vector.tensor_copy(
                                Vsb[:, kti, :, 0:HD],
                                ps_tb[:, 0:128]
                                .rearrange("p (h j) -> p h j", h=HPC))
                        return u

                    return [u0, u1, tr(0), tr(1), tr(2), tr(3)]

                def run_units(units):
                    for u in units:
                        u()

                def emit_tail(o_ps, qc):
                    # scatter the unnormalized O + denominator row for
                    # chunk qc (normalization happens on the receiver);
                    # with the permuted layout, query column r*64+j goes
                    # to receiver r, so the payload is 'on' verbatim.
                    for h in range(HPC):
                        on = B_db.tile([HD + 1, QCH], BF16, tag="on")
                        nc.vector.tensor_copy(on[:], o_ps[h][:])
                        nc.gpsimd.dma_start(
                            out=a2a_ins[qc].bitcast(BF16)[:, h, :, :]
                            .rearrange("r p n -> p r n"),
                            in_=on[:].rearrange("p (r n) -> p r n",
                                               r=NCORES))
                    if parts in ("abc", "real"):
                        if parts == "abc":
                            # timing stand-in: same data, no mesh sync
                            nc.sync.dma_start(out=a2a_outs[qc][:],
                                              in_=a2a_ins[qc][:])
                        else:
                            nc.gpsimd.collective_compute(
                                "AllToAll", mybir.AluOpType.bypass,
                                replica_groups=[list(range(NCORES))],
                                ins=[a2a_ins[qc][:]], outs=[a2a_outs[qc][:]])

                def round_units(qc):
                    # receiver side of round qc, split into a PE-free
                    # prefix (gathers + reciprocal) and a compute unit
                    # (whose psSm slots stay contiguous in emission).
                    box = {}

                    def u_pre():
                        a2a_b = a2a_outs[qc].bitcast(BF16)  # [s, h, 65, 64]
                        dn = B_sm.tile([NHEADS, HD], BF16, tag="dn",
                                       name="dn")
                        nc.sync.dma_start(
                            out=dn[:], in_=a2a_b[:, :, HD, :]
                            .rearrange("s h n -> (s h) n"))
                        ga = B_sm.tile([CH, NCORES, HD], BF16, tag="ga",
                                       name="ga")
                        for h in range(HPC):
                            nc.sync.dma_start(
                                out=ga[h * HD:(h + 1) * HD],
                                in_=a2a_b[:, h, 0:HD, :]
                                .rearrange("s d n -> d s n"))
                        rec = B_sm.tile([NHEADS, HD], BF16, tag="rec",
                                        name="rec")
                        with nc.allow_low_precision(
                                reason="softmax denom recip; bf16 ok"):
                            nc.vector.reciprocal(rec[:], dn[:])
                        box["ga"], box["rec"] = ga, rec

                    def u_main():
                        ga, rec = box["ga"], box["rec"]
                        fac = psSm.tile([CH, QCH], F32, tag="sm",
                                        name="fac")
                        facv = fac.rearrange("p (s n) -> p s n", s=NCORES)
                        for src in range(NCORES):
                            nc.tensor.matmul(facv[:, src, :],
                                             sel[:, src, :], rec[:],
                                             start=True, stop=True)
                        on2 = B_sm.tile([CH, NCORES, HD], BF16, tag="on2",
                                        name="on2")
                        nc.vector.tensor_mul(
                            on2[:].rearrange("p s n -> p (s n)"),
                            fac[:], ga[:].rearrange("p s n -> p (s n)"))
                        for oc in range(2):
                            pp = psSm.tile([CH, QCH], F32, tag="sm",
                                           name="pp")
                            for src in range(NCORES):
                                nc.tensor.matmul(
                                    pp[0:HD, :], on2[:, src, :],
                                    wp[:, src, oc * 512:(oc + 1) * 512],
                                    start=(src == 0),
                                    stop=(src == NCORES - 1))
                            ob = B_db.tile([HD, QCH], F32, tag="ob",
                                           name="ob")
                            nc.vector.tensor_copy(ob[:], pp[0:HD, :])
                            nc.sync.dma_start(
                                out=out_ext[qc * HD:(qc + 1) * HD,
                                            oc * 512:(oc + 1) * 512],
                                in_=ob[:])
                    return u_pre, u_main

                def emit_s(qc, kt):
                    sl = slice(qc * QCH, (qc + 1) * QCH)
                    s_ps = psBig.tile([128, HPC, QCH], F32, tag="big")
                    for h in range(HPC):
                        nc.tensor.matmul(
                            s_ps[:, h, :],
                            Kp[h * HD:(h + 1) * HD,
                               kt * 128:(kt + 1) * 128],
                            Qp[h * HD:(h + 1) * HD, sl],
                            start=True, stop=True,
                            tile_position=(h * HD, 0))
                    p_sb = B_db.tile([128, HPC, QCH], BF16, tag="p")
                    nc.scalar.activation(out=p_sb[:], in_=s_ps[:], func=EXP)
                    return p_sb

                o_tiles = {}

                def emit_o(qc, kt, p_sb):
                    if kt == 0:
                        # allocated here (not at S-emission) so the psO
                        # slot-reuse dependency sees the previous chunk's
                        # tail reads, which are emitted before this point.
                        o0 = psO.tile([HD + 1, QCH], F32, tag="oaug")
                        o1 = psO.tile([HD + 1, QCH], F32, tag="oaug")
                        o_tiles[qc] = [o0, o1]
                    for h in range(HPC):
                        nc.tensor.matmul(
                            o_tiles[qc][h][:], Vsb[:, kt, h, :],
                            p_sb[:, h, :],
                            start=(kt == 0), stop=(kt == NKT - 1))

                # ---- lead-in: block-0 K, Q(0), V(0) straight away.
                # (V transposes cost no PE time; emit them here too.)
                run_units(proj_units(wq, Qp, 0))
                run_units(proj_units(wk, Kp, 0))
                run_units(v_units(0))

                # background emission units, drained between S steps.
                # Order respects data deadlines: O(0, kt) is emitted at
                # step kt+2, S(0, kt) at step kt, Q(qc) before step 16*qc.
                bg = []
                bg += proj_units(wk, Kp, 1)      # K(1): before step 4
                bg += v_units(1)                 # V(1): before step 6
                bg += proj_units(wk, Kp, 2)      # K(2): before step 8
                bg += v_units(2)                 # V(2): before step 10
                bg += proj_units(wk, Kp, 3)      # K(3): before step 12
                bg += v_units(3)                 # V(3): before step 14
                bg += proj_units(wq, Qp, 1)      # Q(1): before step 16
                bg += proj_units(wq, Qp, 2)      # Q(2): before step 32
                bg += proj_units(wq, Qp, 3)      # Q(3): before step 48
                bg.reverse()                     # pop() from the end
                DRAIN = {0: 2, 1: 2, 2: 2, 3: 2}

                if parts == "a":
                    while bg:
                        bg.pop()()
                    return

                # Global software pipeline over the 64 (qc, kt) steps:
                # O-emission runs 2 steps behind S/exp; at a chunk's last
                # k-tile we catch up, pre-emit the next chunk's first S so
                # the exp stream never waits on the tail, then emit the
                # tail (whose reads land before the next chunk's first O
                # allocates the psO slots).
                p_fifo = []          # [(qc, kt, p_sb)] not yet O-consumed
                emitted = set()

                def s_step(step):
                    if step in emitted or step >= NQC * NKT:
                        return
                    emitted.add(step)
                    qc, kt = divmod(step, NKT)
                    p_fifo.append((qc, kt, emit_s(qc, kt)))

                s_only = parts == "s"
                do_c = parts in ("abc", "real")
                for step in range(NQC * NKT):
                    qc, kt = divmod(step, NKT)
                    s_step(step)
                    if kt == NKT - 1:
                        while len(p_fifo) > 1:
                            if not s_only:
                                emit_o(*p_fifo.pop(0))
                            else:
                                p_fifo.pop(0)
                        s_step(step + 1)
                        if not s_only:
                            emit_o(*p_fifo.pop(0))
                            emit_tail(o_tiles[qc], qc)
                            if do_c and qc < NQC - 1:
                                pre, main = round_units(qc)
                                bg.append(pre)
                                bg.insert(0, main)
                        else:
                            p_fifo.pop(0)
                    else:
                        while len(p_fifo) > 2:
                            if not s_only:
                                emit_o(*p_fifo.pop(0))
                            else:
                                p_fifo.pop(0)
                        for _ in range(DRAIN[qc]):
                            if bg:
                                bg.pop()()
                while bg:
                    bg.pop()()
                if do_c and not s_only:
                    pre, main = round_units(NQC - 1)
                    pre()
                    main()

        with tc.tile_pool(name="persist", bufs=1) as P1:
            Qp = P1.tile([CH, SEQ], BF16, tag="Qp")
            Kp = P1.tile([CH, SEQ], BF16, tag="Kp")
            Vsb = P1.tile([128, NKT, HPC, HD + 1], BF16, tag="Vsb")
            sel = P1.tile([NHEADS, NCORES, 128], BF16, tag="sel")
            xt = P1.tile([128, DCH, SEQ], BF16, tag="xt")
            use_c = parts in ("abc", "real")
            wp = (P1.tile([128, DCH, DIM], BF16, tag="wp", name="wp")
                  if use_c else None)
            if nrep == 1:
                stage_ab(Qp, Kp, Vsb, sel, xt, wp, parts)
            else:
                # timing build: loop the whole body; "abc" replaces each
                # collective with a local DMA (a collective inside a
                # For_i desyncs the mesh).
                with tc.For_i(0, nrep, 1) as _i:
                    stage_ab(Qp, Kp, Vsb, sel, xt, wp, parts)

    nc.compile()
    return nc


def _get_nc(nrep=1, n_cores=NCORES, with_c=True, parts="real"):
    key = ("nc", nrep, n_cores, with_c, parts)
    if key not in _CACHE:
        _CACHE[key] = _build(nrep, n_cores, with_c, parts)
    return _CACHE[key]


def _prep_in_maps(x, wq, wk, wv, wp):
    import ml_dtypes

    bf16 = ml_dtypes.bfloat16
    x2 = np.ascontiguousarray(np.asarray(x, np.float32).reshape(SEQ, DIM))
    xt = np.ascontiguousarray(x2.T)
    # permute the sequence so kernel chunk qc holds 64 query rows for
    # every receiver: kernel col p <- original col 4*(p%512) + p//512
    p_idx = np.arange(SEQ)
    perm = 4 * (p_idx % QCH) + p_idx // QCH
    xt = np.ascontiguousarray(xt[:, perm]).astype(bf16)
    wq = np.asarray(wq, np.float64)
    wk = np.asarray(wk, np.float64)
    wv = np.asarray(wv, np.float64)
    wp = np.asarray(wp, np.float32)
    cos2, s2 = _rope_tables()
    scale = 1.0 / np.sqrt(HD)
    wq = wq * scale
    ck = np.ascontiguousarray(cos2[:, perm]).astype(bf16)
    sk = np.ascontiguousarray(s2[:, perm]).astype(bf16)
    r2t = _r2t().astype(bf16)
    sel = np.zeros((NCORES, NHEADS, 128), np.float32)
    for src in range(NCORES):
        for p in range(128):
            sel[src, src * HPC + p // HD, p] = 1.0
    sel = sel.astype(bf16)
    wpt = np.ascontiguousarray(wp.T)
    def wlay(w_t):
        # [DIM, CH] -> [128 part, DCH*CH] so the DMA is contiguous
        return np.ascontiguousarray(
            w_t.reshape(DCH, 128, CH).transpose(1, 0, 2).reshape(
                128, DCH * CH)).astype(bf16)

    wp_l = np.ascontiguousarray(
        wpt.reshape(DCH, 128, DIM).transpose(1, 0, 2).reshape(
            128, DCH * DIM)).astype(bf16)
    maps = []
    for c in range(NCORES):
        ch = slice(c * CH, (c + 1) * CH)
        maps.append({
            "xt": xt,
            "wq_t": wlay(np.ascontiguousarray(wq[ch, :].T)),
            "wk_t": wlay(np.ascontiguousarray(wk[ch, :].T)),
            "wv_t": wlay(np.ascontiguousarray(wv[ch, :].T)),
            "wp_t": wp_l,
            "cos_k": ck, "sin_k": sk,
            "r2t": r2t, "sel": sel,
        })
    return maps


def kernel(x, wq, wk, wv, wp):
    from concourse.bass_utils import run_bass_kernel_spmd

    nc = _get_nc(1)
    maps = _prep_in_maps(x, wq, wk, wv, wp)
    res = run_bass_kernel_spmd(nc, maps, list(range(NCORES))).results
    npc = SEQ // NCORES
    nn = np.arange(npc)
    rowmap = (nn % NQC) * HD + nn // NQC
    out = np.concatenate([res[c]["out"][rowmap] for c in range(NCORES)],
                         axis=0)
    return out.reshape(1, SEQ, DIM).astype(np.float32)


# revision 37
# speedup vs baseline: 1.0321x; 1.0321x over previous
"""Trainium2 Bass kernel for multi-head attention with RoPE (nn_Attention).

Reference computation (B=1, N=2048, D=1024, 16 heads, hd=64):
    q = x @ wq.T; k = x @ wk.T; v = x @ wv.T      (reshaped to heads)
    q, k = rope(q), rope(k)
    out = softmax(q k^T / sqrt(hd)) v              (non-causal, full)
    return (out reshaped) @ wp.T

Sharding: tensor-parallel over heads — each of the 8 cores owns 2 heads for
QKV projection + SDPA, then an AllToAll redistributes the attention output
so each core computes the final projection for its 256 sequence rows with
the full wp.

v2: all matmul operands bf16 (host-precast), x streamed in 4 column blocks
so attention starts after the first block, ScalarE reserved exclusively for
exp, V transposed via the DMA xbar, wp prefetched during attention.

Self-contained: only imports numpy + the concourse stack available in the
execution environment. kernel(**inputs) takes the full unsharded inputs and
returns the full output.
"""
import numpy as np

DIM = 1024
NHEADS = 16
HD = 64
SEQ = 2048
NCORES = 8
ROPE_BASE = 10000.0
HPC = NHEADS // NCORES      # heads per core = 2
CH = HPC * HD               # channels per core = 128
QCH = 512                   # q-chunk (free dim of S/P tiles)
NQC = SEQ // QCH            # 4
NKT = SEQ // 128            # 16 k-tiles
DCH = DIM // 128            # 8 contraction chunks
NBLK = 4                    # x column blocks (512 seq cols each)

_CACHE = {}


def _rope_tables():
    inv = 1.0 / (ROPE_BASE ** (np.arange(0, HD, 2, dtype=np.float64) / HD))
    t = np.arange(SEQ, dtype=np.float64)
    freqs = np.outer(t, inv)                      # [SEQ, 32]
    emb = np.concatenate([freqs, freqs], 1)       # [SEQ, 64]
    cosT = np.cos(emb).T                          # [64, SEQ]
    sinT = np.sin(emb).T
    sig = (np.arange(HD) + 32) % HD
    sT = sinT[sig]                                # shifted sin
    cos2 = np.concatenate([cosT, cosT], 0)        # [128, SEQ] (2 heads)
    s2 = np.concatenate([sT, sT], 0)
    return cos2, s2


def _r2t():
    # rotate-half matrix R (per head), block-diagonal over the 2 heads; we
    # pass R2.T as the stationary matmul operand.
    R = np.zeros((HD, HD), np.float64)
    for j in range(32):
        R[j, j + 32] = -1.0
        R[j + 32, j] = 1.0
    R2 = np.zeros((CH, CH), np.float64)
    R2[0:HD, 0:HD] = R
    R2[HD:CH, HD:CH] = R
    return np.ascontiguousarray(R2.T)


def _build(nrep=1, n_cores=NCORES, with_c=True, parts="real"):
    import concourse.mybir as mybir
    import concourse.tile as tile
    from concourse import bacc
    from concourse.masks import make_identity

    F32 = mybir.dt.float32
    F32R = mybir.dt.float32r
    BF16 = mybir.dt.bfloat16
    EXP = mybir.ActivationFunctionType.Exp

    nc = bacc.Bacc("TRN2", target_bir_lowering=False, debug=False,
                   num_devices=n_cores)

    xt_ext = nc.dram_tensor("xt", [DIM, SEQ], BF16, kind="ExternalInput")
    wq_ext = nc.dram_tensor("wq_t", [128, DCH * CH], BF16,
                            kind="ExternalInput")
    wk_ext = nc.dram_tensor("wk_t", [128, DCH * CH], BF16,
                            kind="ExternalInput")
    wv_ext = nc.dram_tensor("wv_t", [128, DCH * CH], BF16,
                            kind="ExternalInput")
    wp_ext = nc.dram_tensor("wp_t", [128, DCH * DIM], BF16,
                            kind="ExternalInput")
    ck_ext = nc.dram_tensor("cos_k", [CH, SEQ], BF16, kind="ExternalInput")
    sk_ext = nc.dram_tensor("sin_k", [CH, SEQ], BF16, kind="ExternalInput")
    r2t_ext = nc.dram_tensor("r2t", [CH, CH], BF16, kind="ExternalInput")
    sel_ext = nc.dram_tensor("sel", [NCORES, NHEADS, 128], BF16,
                             kind="ExternalInput")
    out_ext = nc.dram_tensor("out", [SEQ // NCORES, DIM], F32,
                             kind="ExternalOutput")
    # Collective payload is bf16 on the wire, but the collective machinery
    # mishandles sub-4-byte dtypes, so the DRAM tensors are declared f32
    # (half the elements) and DMAs bitcast at the boundary. The sequence
    # axis is host-permuted so each q-chunk contains 64 rows for EVERY
    # receiver; one small AllToAll runs per chunk, hiding the output
    # projection under later chunks' attention. Each (dst, head) slot
    # carries the unnormalized O (64 rows) plus the softmax denominator
    # row (row 64); the receiver normalizes.
    a2a_ins = [nc.dram_tensor(f"a2a_in{qc}", [NCORES, HPC, HD + 1, 32], F32)
               for qc in range(NQC)]
    a2a_outs = [nc.dram_tensor(f"a2a_out{qc}", [NCORES, HPC, HD + 1, 32], F32)
                for qc in range(NQC)]

    with tile.TileContext(nc) as tc:

        def stage_ab(Qp, Kp, Vsb, sel, xt, wp, parts="ab"):
            with (
                tc.tile_pool(name="stA", bufs=1) as A_sb,
                tc.tile_pool(name="stA2", bufs=2) as A_db,
                tc.tile_pool(name="psBig", bufs=2, space="PSUM") as psBig,
                tc.tile_pool(name="psSm", bufs=2, space="PSUM") as psSm,
                tc.tile_pool(name="psO", bufs=2, space="PSUM") as psO,
                tc.tile_pool(name="stB", bufs=4) as B_db,
                tc.tile_pool(name="stBs", bufs=3) as B_sm,
            ):
                aux1 = A_sb.tile([128, HD], F32, tag="aux1")
                nc.vector.memset(aux1[:], 1.0)
                identf = A_sb.tile([128, 128], F32, tag="identf")
                make_identity(nc, identf[:])
                ident = A_sb.tile([128, 128], BF16, tag="ident")
                nc.vector.tensor_copy(ident[:], identf[:])
                # warm the ACT exp table at t=0 so the first real exp
                # doesn't eat the ~2.7us table load.
                warm = A_sb.tile([1, 8], F32, tag="warm")
                nc.scalar.activation(out=warm[:], in_=aux1[0:1, 0:8], func=EXP)
                nc.vector.tensor_copy(
                    Vsb[:, :, :, HD],
                    aux1[:, 0:NKT * HPC].rearrange("p (k h) -> p k h", h=HPC))

                # ---- input DMAs (all bf16, single stream on sync queue),
                # ordered so the block-0 critical path (wk, xt block 0,
                # rope tables, wq) lands first.
                wk = A_sb.tile([128, DCH, CH], BF16, tag="wk")
                wq = A_sb.tile([128, DCH, CH], BF16, tag="wq")
                wv = A_sb.tile([128, DCH, CH], BF16, tag="wv")
                r2t = A_sb.tile([CH, CH], BF16, tag="r2t")
                ck = A_sb.tile([CH, SEQ], BF16, tag="ck")
                sk = A_sb.tile([CH, SEQ], BF16, tag="sk")
                xt_r = xt_ext.rearrange("(c p) n -> p c n", p=128)
                b0 = slice(0, QCH)
                nc.sync.dma_start(
                    out=wq[:], in_=wq_ext.rearrange("p (c j) -> p c j", j=CH))
                nc.sync.dma_start(
                    out=wk[:], in_=wk_ext.rearrange("p (c j) -> p c j", j=CH))
                for d in range(DCH):
                    nc.sync.dma_start(out=xt[:, d, b0], in_=xt_r[:, d, b0])
                nc.sync.dma_start(out=r2t[:], in_=r2t_ext[:])
                if wp is not None:
                    nc.gpsimd.dma_start(
                        out=sel[:], in_=sel_ext.rearrange("s r p -> r s p"))
                nc.sync.dma_start(out=sk[:, b0], in_=sk_ext[:, b0])
                nc.sync.dma_start(out=ck[:, b0], in_=ck_ext[:, b0])
                nc.sync.dma_start(
                    out=wv[:], in_=wv_ext.rearrange("p (c j) -> p c j", j=CH))
                rest = slice(QCH, SEQ)
                nc.sync.dma_start(out=sk[:, rest], in_=sk_ext[:, rest])
                nc.sync.dma_start(out=ck[:, rest], in_=ck_ext[:, rest])
                for b in range(1, NBLK):
                    sl = slice(b * QCH, (b + 1) * QCH)
                    nc.sync.dma_start(out=xt[:, :, sl], in_=xt_r[:, :, sl])
                if wp is not None:
                    # prefetch wp for stage C behind the x blocks, on the
                    # gpsimd queue so it doesn't delay the x streams
                    nc.gpsimd.dma_start(
                        out=wp[:],
                        in_=wp_ext.rearrange("p (s o) -> p s o", o=DIM))

                def rope_to(acc_ps, dst, sl, width):
                    # dst[:, sl] = acc*cos + R2T @ (acc*sin), overwriting
                    # acc_ps in place for the rotation matmul.
                    qs = A_db.tile([CH, width], BF16, tag="qs")
                    nc.vector.tensor_mul(qs[:], acc_ps[:], sk[:, sl])
                    qct = A_db.tile([CH, width], BF16, tag="qct")
                    nc.vector.tensor_mul(qct[:], acc_ps[:], ck[:, sl])
                    nc.tensor.matmul(acc_ps[:], r2t[:], qs[:],
                                     start=True, stop=True)
                    nc.vector.tensor_add(dst[:, sl], qct[:], acc_ps[:])

                def proj_units(w_sb, dst, b):
                    # split a 1024-contraction projection + rope into 3
                    # emission units so it can be paced between S steps.
                    sl = slice(b * QCH, (b + 1) * QCH)
                    box = {}

                    def u0():
                        box["ps"] = psSm.tile([CH, QCH], F32, tag="sm", name="ps")
                        for d in range(4):
                            nc.tensor.matmul(box["ps"][:], w_sb[:, d, :],
                                             xt[:, d, sl],
                                             start=(d == 0), stop=False)

                    def u1():
                        for d in range(4, DCH):
                            nc.tensor.matmul(box["ps"][:], w_sb[:, d, :],
                                             xt[:, d, sl],
                                             start=False, stop=(d == DCH - 1))

                    def u2():
                        rope_to(box["ps"], dst, sl, QCH)

                    return [u0, u1, u2]

                def v_units(b):
                    sl = slice(b * QCH, (b + 1) * QCH)
                    box = {}

                    def u0():
                        box["ps"] = psSm.tile([CH, QCH], F32, tag="sm", name="ps")
                        for d in range(4):
                            nc.tensor.matmul(box["ps"][:], wv[:, d, :],
                                             xt[:, d, sl],
                                             start=(d == 0), stop=False)

                    def u1():
                        for d in range(4, DCH):
                            nc.tensor.matmul(box["ps"][:], wv[:, d, :],
                                             xt[:, d, sl],
                                             start=False, stop=(d == DCH - 1))
                        vt = B_db.tile([CH, QCH], BF16, tag="vt",
                                       name="vt")
                        nc.vector.tensor_copy(vt[:], box["ps"][:])
                        box["vt"] = vt

                    def tr(i):
                        def u():
                            kti = b * (QCH // 128) + i
                            ps_t = psSm.tile([CH, QCH], F32, tag="sm",
                                             name="ps_t")
                            ps_tb = ps_t.bitcast(BF16)
                            nc.tensor.transpose(
                                ps_tb[:, 0:128],
                                box["vt"][:, i * 128:(i + 1) * 128],
                                ident[:])
                            nc.vector.tensor_copy(
                                Vsb[:, kti, :, 0:HD],
                                ps_tb[:, 0:128]
                                .rearrange("p (h j) -> p h j", h=HPC))
                        return u

                    return [u0, u1, tr(0), tr(1), tr(2), tr(3)]

                def run_units(units):
                    for u in units:
                        u()

                def emit_tail(o_ps, qc):
                    # scatter the unnormalized O + denominator row for
                    # chunk qc (normalization happens on the receiver);
                    # with the permuted layout, query column r*64+j goes
                    # to receiver r, so the payload is 'on' verbatim.
                    for h in range(HPC):
                        on = B_db.tile([HD + 1, QCH], BF16, tag="on")
                        nc.vector.tensor_copy(on[:], o_ps[h][:])
                        nc.gpsimd.dma_start(
                            out=a2a_ins[qc].bitcast(BF16)[:, h, :, :]
                            .rearrange("r p n -> p r n"),
                            in_=on[:].rearrange("p (r n) -> p r n",
                                               r=NCORES))
                    if parts in ("abc", "abn", "real"):
                        if parts == "abn":
                            pass  # timing mode: no data movement at all
                        elif parts == "abc":
                            # timing stand-in: same data, no mesh sync
                            nc.sync.dma_start(out=a2a_outs[qc][:],
                                              in_=a2a_ins[qc][:])
                        else:
                            nc.gpsimd.collective_compute(
                                "AllToAll", mybir.AluOpType.bypass,
                                replica_groups=[list(range(NCORES))],
                                ins=[a2a_ins[qc][:]], outs=[a2a_outs[qc][:]])

                a2a_srcs = a2a_ins if parts == "abn" else a2a_outs

                def round_units(qc):
                    # receiver side of round qc, split into a PE-free
                    # prefix (gathers + reciprocal) and a compute unit
                    # (whose psSm slots stay contiguous in emission).
                    box = {}

                    def u_pre():
                        a2a_b = a2a_srcs[qc].bitcast(BF16)  # [s, h, 65, 64]
                        dn = B_sm.tile([NHEADS, HD], BF16, tag="dn",
                                       name="dn")
                        nc.sync.dma_start(
                            out=dn[:], in_=a2a_b[:, :, HD, :]
                            .rearrange("s h n -> (s h) n"))
                        ga = B_sm.tile([CH, NCORES, HD], BF16, tag="ga",
                                       name="ga")
                        for h in range(HPC):
                            nc.sync.dma_start(
                                out=ga[h * HD:(h + 1) * HD],
                                in_=a2a_b[:, h, 0:HD, :]
                                .rearrange("s d n -> d s n"))
                        rec = B_sm.tile([NHEADS, HD], BF16, tag="rec",
                                        name="rec")
                        with nc.allow_low_precision(
                                reason="softmax denom recip; bf16 ok"):
                            nc.vector.reciprocal(rec[:], dn[:])
                        box["ga"], box["rec"] = ga, rec

                    def u_main():
                        ga, rec = box["ga"], box["rec"]
                        fac = psSm.tile([CH, QCH], F32, tag="sm",
                                        name="fac")
                        facv = fac.rearrange("p (s n) -> p s n", s=NCORES)
                        for src in range(NCORES):
                            nc.tensor.matmul(facv[:, src, :],
                                             sel[:, src, :], rec[:],
                                             start=True, stop=True)
                        on2 = B_sm.tile([CH, NCORES, HD], BF16, tag="on2",
                                        name="on2")
                        nc.vector.tensor_mul(
                            on2[:].rearrange("p s n -> p (s n)"),
                            fac[:], ga[:].rearrange("p s n -> p (s n)"))
                        for oc in range(2):
                            pp = psSm.tile([CH, QCH], F32, tag="sm",
                                           name="pp")
                            for src in range(NCORES):
                                nc.tensor.matmul(
                                    pp[0:HD, :], on2[:, src, :],
                                    wp[:, src, oc * 512:(oc + 1) * 512],
                                    start=(src == 0),
                                    stop=(src == NCORES - 1))
                            ob = B_db.tile([HD, QCH], F32, tag="ob",
                                           name="ob")
                            nc.vector.tensor_copy(ob[:], pp[0:HD, :])
                            nc.sync.dma_start(
                                out=out_ext[qc * HD:(qc + 1) * HD,
                                            oc * 512:(oc + 1) * 512],
                                in_=ob[:])
                    return u_pre, u_main

                def emit_s(qc, kt):
                    sl = slice(qc * QCH, (qc + 1) * QCH)
                    s_ps = psBig.tile([128, HPC, QCH], F32, tag="big")
                    for h in range(HPC):
                        nc.tensor.matmul(
                            s_ps[:, h, :],
                            Kp[h * HD:(h + 1) * HD,
                               kt * 128:(kt + 1) * 128],
                            Qp[h * HD:(h + 1) * HD, sl],
                            start=True, stop=True,
                            tile_position=(h * HD, 0))
                    p_sb = B_db.tile([128, HPC, QCH], BF16, tag="p")
                    nc.scalar.activation(out=p_sb[:], in_=s_ps[:], func=EXP)
                    return p_sb

                o_tiles = {}

                def emit_o(qc, kt, p_sb):
                    if kt == 0:
                        # allocated here (not at S-emission) so the psO
                        # slot-reuse dependency sees the previous chunk's
                        # tail reads, which are emitted before this point.
                        o0 = psO.tile([HD + 1, QCH], F32, tag="oaug")
                        o1 = psO.tile([HD + 1, QCH], F32, tag="oaug")
                        o_tiles[qc] = [o0, o1]
                    for h in range(HPC):
                        nc.tensor.matmul(
                            o_tiles[qc][h][:], Vsb[:, kt, h, :],
                            p_sb[:, h, :],
                            start=(kt == 0), stop=(kt == NKT - 1))

                # ---- lead-in: block-0 K, Q(0), V(0) straight away.
                # (V transposes cost no PE time; emit them here too.)
                run_units(proj_units(wq, Qp, 0))
                run_units(proj_units(wk, Kp, 0))
                run_units(v_units(0))

                # background emission units, drained between S steps.
                # Order respects data deadlines: O(0, kt) is emitted at
                # step kt+2, S(0, kt) at step kt, Q(qc) before step 16*qc.
                bg = []
                bg += proj_units(wk, Kp, 1)      # K(1): before step 4
                bg += v_units(1)                 # V(1): before step 6
                bg += proj_units(wk, Kp, 2)      # K(2): before step 8
                bg += v_units(2)                 # V(2): before step 10
                bg += proj_units(wk, Kp, 3)      # K(3): before step 12
                bg += v_units(3)                 # V(3): before step 14
                bg += proj_units(wq, Qp, 1)      # Q(1): before step 16
                bg += proj_units(wq, Qp, 2)      # Q(2): before step 32
                bg += proj_units(wq, Qp, 3)      # Q(3): before step 48
                bg.reverse()                     # pop() from the end
                DRAIN = {0: 2, 1: 2, 2: 2, 3: 2}

                if parts == "a":
                    while bg:
                        bg.pop()()
                    return

                # Global software pipeline over the 64 (qc, kt) steps:
                # O-emission runs 2 steps behind S/exp; at a chunk's last
                # k-tile we catch up, pre-emit the next chunk's first S so
                # the exp stream never waits on the tail, then emit the
                # tail (whose reads land before the next chunk's first O
                # allocates the psO slots).
                p_fifo = []          # [(qc, kt, p_sb)] not yet O-consumed
                emitted = set()

                def s_step(step):
                    if step in emitted or step >= NQC * NKT:
                        return
                    emitted.add(step)
                    qc, kt = divmod(step, NKT)
                    p_fifo.append((qc, kt, emit_s(qc, kt)))

                s_only = parts == "s"
                do_c = parts in ("abc", "abn", "real")
                for step in range(NQC * NKT):
                    qc, kt = divmod(step, NKT)
                    s_step(step)
                    if kt == NKT - 1:
                        while len(p_fifo) > 1:
                            if not s_only:
                                emit_o(*p_fifo.pop(0))
                            else:
                                p_fifo.pop(0)
                        s_step(step + 1)
                        if not s_only:
                            emit_o(*p_fifo.pop(0))
                            emit_tail(o_tiles[qc], qc)
                            if do_c and qc < NQC - 1:
                                pre, main = round_units(qc)
                                bg.append(pre)
                                bg.insert(0, main)
                        else:
                            p_fifo.pop(0)
                    else:
                        while len(p_fifo) > 2:
                            if not s_only:
                                emit_o(*p_fifo.pop(0))
                            else:
                                p_fifo.pop(0)
                        # hold background work out of the first steps of
                        # chunks 1+ so round groups land mid-chunk, away
                        # from the boundary's S/O catch-up burst.
                        n_drain = 0 if (qc >= 1 and kt < 4) else DRAIN[qc]
                        for _ in range(n_drain):
                            if bg:
                                bg.pop()()
                while bg:
                    bg.pop()()
                if do_c and not s_only:
                    pre, main = round_units(NQC - 1)
                    pre()
                    main()

        with tc.tile_pool(name="persist", bufs=1) as P1:
            Qp = P1.tile([CH, SEQ], BF16, tag="Qp")
            Kp = P1.tile([CH, SEQ], BF16, tag="Kp")
            Vsb = P1.tile([128, NKT, HPC, HD + 1], BF16, tag="Vsb")
            sel = P1.tile([NHEADS, NCORES, 128], BF16, tag="sel")
            xt = P1.tile([128, DCH, SEQ], BF16, tag="xt")
            use_c = parts in ("abc", "abn", "real")
            wp = (P1.tile([128, DCH, DIM], BF16, tag="wp", name="wp")
                  if use_c else None)
            if nrep == 1:
                stage_ab(Qp, Kp, Vsb, sel, xt, wp, parts)
            else:
                # timing build: loop the whole body; "abc" replaces each
                # collective with a local DMA (a collective inside a
                # For_i desyncs the mesh).
                with tc.For_i(0, nrep, 1) as _i:
                    stage_ab(Qp, Kp, Vsb, sel, xt, wp, parts)

    nc.compile()
    return nc


def _get_nc(nrep=1, n_cores=NCORES, with_c=True, parts="real"):
    key = ("nc", nrep, n_cores, with_c, parts)
    if key not in _CACHE:
        _CACHE[key] = _build(nrep, n_cores, with_c, parts)
    return _CACHE[key]


def _prep_in_maps(x, wq, wk, wv, wp):
    import ml_dtypes

    bf16 = ml_dtypes.bfloat16
    x2 = np.ascontiguousarray(np.asarray(x, np.float32).reshape(SEQ, DIM))
    xt = np.ascontiguousarray(x2.T)
    # permute the sequence so kernel chunk qc holds 64 query rows for
    # every receiver: kernel col p <- original col 4*(p%512) + p//512
    p_idx = np.arange(SEQ)
    perm = 4 * (p_idx % QCH) + p_idx // QCH
    xt = np.ascontiguousarray(xt[:, perm]).astype(bf16)
    wq = np.asarray(wq, np.float64)
    wk = np.asarray(wk, np.float64)
    wv = np.asarray(wv, np.float64)
    wp = np.asarray(wp, np.float32)
    cos2, s2 = _rope_tables()
    scale = 1.0 / np.sqrt(HD)
    wq = wq * scale
    ck = np.ascontiguousarray(cos2[:, perm]).astype(bf16)
    sk = np.ascontiguousarray(s2[:, perm]).astype(bf16)
    r2t = _r2t().astype(bf16)
    sel = np.zeros((NCORES, NHEADS, 128), np.float32)
    for src in range(NCORES):
        for p in range(128):
            sel[src, src * HPC + p // HD, p] = 1.0
    sel = sel.astype(bf16)
    wpt = np.ascontiguousarray(wp.T)
    def wlay(w_t):
        # [DIM, CH] -> [128 part, DCH*CH] so the DMA is contiguous
        return np.ascontiguousarray(
            w_t.reshape(DCH, 128, CH).transpose(1, 0, 2).reshape(
                128, DCH * CH)).astype(bf16)

    wp_l = np.ascontiguousarray(
        wpt.reshape(DCH, 128, DIM).transpose(1, 0, 2).reshape(
            128, DCH * DIM)).astype(bf16)
    maps = []
    for c in range(NCORES):
        ch = slice(c * CH, (c + 1) * CH)
        maps.append({
            "xt": xt,
            "wq_t": wlay(np.ascontiguousarray(wq[ch, :].T)),
            "wk_t": wlay(np.ascontiguousarray(wk[ch, :].T)),
            "wv_t": wlay(np.ascontiguousarray(wv[ch, :].T)),
            "wp_t": wp_l,
            "cos_k": ck, "sin_k": sk,
            "r2t": r2t, "sel": sel,
        })
    return maps


def kernel(x, wq, wk, wv, wp):
    from concourse.bass_utils import run_bass_kernel_spmd

    nc = _get_nc(1)
    maps = _prep_in_maps(x, wq, wk, wv, wp)
    res = run_bass_kernel_spmd(nc, maps, list(range(NCORES))).results
    npc = SEQ // NCORES
    nn = np.arange(npc)
    rowmap = (nn % NQC) * HD + nn // NQC
    out = np.concatenate([res[c]["out"][rowmap] for c in range(NCORES)],
                         axis=0)
    return out.reshape(1, SEQ, DIM).astype(np.float32)
